# revision 30
# baseline (speedup 1.0000x reference)
"""AdaptiveInput (adaptive embedding) kernel for 8 TRN2 NeuronCores.

Strategy: data-parallel over tokens (each core takes one batch row of 4096
tokens, embedding tables replicated). The host does only integer index
bookkeeping (compaction of tokens by cluster / vocab sub-range); every
float is touched exclusively on-device:

  per core:  dma_gather rows from DRAM tables -> SBUF (f32)
             PE-transpose each 128-token tile -> PSUM -> bf16 lhsT
             matmul vs bf16 projection weights -> PSUM f32 [128, 1024]
             stage bf16 rows -> dma_scatter_add to out_j[4097, 1024]
             (row 4096 is a trash row: padding slots land there; real
              rows are written exactly once onto the zero-initialized
              outputs, so scatter-ADD == assignment; host widens to f32)

Scatter chunks are group-aligned and rotate across 4 output tensors so
successive scatters have no WAW dependency and their transfers overlap
compute; the host merges the disjoint row sets (unshard-style reassembly).

dma_gather / dma_scatter_add use int16 indices wrapped in 16 partitions,
so vocab ranges larger than 32767 rows are split into sub-range groups.
tail2 rows are only 64B (< the 256B descriptor minimum), so tail2 is
gathered in quad-row chunks (idx = row // 4, 256B) and the unwanted
sub-rows are zeroed after the transpose with a host-provided mask; the
matmul then runs against a 4x-stacked tail_lin2 so the zeroed lanes
contribute nothing.
"""
import sys

if "/opt/trn_rl_repo" not in sys.path:
    sys.path.insert(0, "/opt/trn_rl_repo")

import numpy as np

import concourse.bass as bass
import concourse.tile as tile
from concourse import bacc, mybir
from concourse.bass_utils import run_bass_kernel_spmd

# --- problem constants (hardcoded; kernel.py must be self-contained) ---
N_CORES = 8
N_TOK = 4096                    # tokens per core
D = 1024                        # output feature dim
CUTOFFS = [0, 10000, 60000, 190000, 250000]
HS = [1024, 256, 64, 16]        # embedding width per cluster
SUBRANGE = 32768                # int16 index limit for dma_gather
TRASH_ROW = N_TOK               # padding scatter target
STAGE_TILES = 4                 # output tiles per scatter chunk
N_OUT = 4                       # rotating output tensors
OUT_BF16 = True                # stage/scatter outputs in bf16 (host widens)

F32 = mybir.dt.float32
BF16 = mybir.dt.bfloat16
I16 = mybir.dt.int16


def _plan_groups(tokens_all):
    """Split tokens into gather groups; return group meta + per-core slot data.

    Groups: [head] + [t0 x2 subranges] + [t1 x4 subranges] + [t2-quad].
    Each group g gets cap_g = roundup(max_core_count, 128) slots.
    Slot i of a group <-> (partition i%128, chunk i//128) in SBUF tiles.
    """
    groups = []
    groups.append(dict(cluster=0, lo=0, hi=CUTOFFS[1], quad=False))
    for lo in range(0, CUTOFFS[2] - CUTOFFS[1], SUBRANGE):
        hi = min(lo + SUBRANGE, CUTOFFS[2] - CUTOFFS[1])
        groups.append(dict(cluster=1, lo=CUTOFFS[1] + lo, hi=CUTOFFS[1] + hi, quad=False))
    for lo in range(0, CUTOFFS[3] - CUTOFFS[2], SUBRANGE):
        hi = min(lo + SUBRANGE, CUTOFFS[3] - CUTOFFS[2])
        groups.append(dict(cluster=2, lo=CUTOFFS[2] + lo, hi=CUTOFFS[2] + hi, quad=False))
    groups.append(dict(cluster=3, lo=CUTOFFS[3], hi=CUTOFFS[4], quad=True))

    per_core = []
    for i in range(N_CORES):
        t = tokens_all[i]
        cg = []
        for g in groups:
            sel = np.nonzero((t >= g["lo"]) & (t < g["hi"]))[0]
            loc = t[sel] - g["lo"]
            cg.append((sel.astype(np.int64), loc.astype(np.int64)))
        per_core.append(cg)

    for gi, g in enumerate(groups):
        mx = max(len(per_core[i][gi][0]) for i in range(N_CORES))
        g["mx"] = -(-max(1, mx) // 128) * 128
        g["cap"] = max(128, -(-mx // 128) * 128)
        g["C"] = g["cap"] // 128

    # scatter chunk plan: chunks never span groups; exact idx counts
    chunks = []
    for gi, g in enumerate(groups):
        for t0 in range(0, g["C"], STAGE_TILES):
            ntc = min(STAGE_TILES, g["C"] - t0)
            n_idx = ntc * 128
            chunks.append(dict(gi=gi, t0=t0, ntc=ntc, n_idx=n_idx))
    return groups, per_core, chunks


def _wrap16(vals, cap, pad):
    """vals -> int16 [128, cap//16]: entry i at [i%16, i//16], replicated x8."""
    m = np.full((16, cap // 16), pad, np.int16)
    n = len(vals)
    m[np.arange(n) % 16, np.arange(n) // 16] = vals.astype(np.int16)
    return np.tile(m, (8, 1))


def _build_graph(groups, chunks, C2):
    S_tot = sum(g["cap"] // 16 for g in groups)
    n_tiles_tot = sum(g["C"] for g in groups)
    n_groups = len(groups)

    nc = bacc.Bacc("TRN2", target_bir_lowering=False, debug=False,
                   num_devices=N_CORES, num_swdge_queues=4)

    p_emb = [
        nc.dram_tensor("head_emb", [CUTOFFS[1], 1024], F32, kind="ExternalInput").ap(),
        nc.dram_tensor("tail_emb0", [CUTOFFS[2] - CUTOFFS[1], 256], F32, kind="ExternalInput").ap(),
        nc.dram_tensor("tail_emb1", [CUTOFFS[3] - CUTOFFS[2], 64], F32, kind="ExternalInput").ap(),
        nc.dram_tensor("tail_emb2", [CUTOFFS[4] - CUTOFFS[3], 16], F32, kind="ExternalInput").ap(),
    ]
    p_hwT = nc.dram_tensor("head_wT", [1024, 1024], F32, kind="ExternalInput").ap()
    p_l0 = nc.dram_tensor("tail_lin0", [256, 1024], F32, kind="ExternalInput").ap()
    p_l1 = nc.dram_tensor("tail_lin1", [64, 1024], F32, kind="ExternalInput").ap()
    p_l2 = nc.dram_tensor("tail_lin2", [16, 1024], F32, kind="ExternalInput").ap()
    p_gidx = nc.dram_tensor("gidx", [128, S_tot], I16, kind="ExternalInput").ap()
    p_spos = nc.dram_tensor("spos", [128, S_tot], I16, kind="ExternalInput").ap()
    p_mask = nc.dram_tensor("maskT2", [64, C2 * 128], F32, kind="ExternalInput").ap()
    p_ident = nc.dram_tensor("ident", [128, 128], F32, kind="ExternalInput").ap()
    out_dt = BF16 if OUT_BF16 else F32
    p_out = [
        nc.dram_tensor(f"out{j}", [N_TOK + 1, D], out_dt, kind="ExternalOutput").ap()
        for j in range(N_OUT)
    ]
    p_l2x4 = nc.dram_tensor("l2x4", [64, 1024], F32).ap()  # internal bounce

    with tile.TileContext(nc) as tc:
        from contextlib import ExitStack
        with ExitStack() as ctx:
            cpool = ctx.enter_context(tc.tile_pool(name="const", bufs=1))
            wstg = ctx.enter_context(tc.tile_pool(name="wstg", bufs=2))
            xgpool = ctx.enter_context(tc.tile_pool(name="xg", bufs=1))
            xtpool = ctx.enter_context(tc.tile_pool(name="xt", bufs=4))
            stpool = ctx.enter_context(tc.tile_pool(name="stage", bufs=6))
            pt_pool = ctx.enter_context(tc.tile_pool(name="ptp", bufs=2, space="PSUM"))
            po_pool = ctx.enter_context(tc.tile_pool(name="pop", bufs=3, space="PSUM"))

            ident = cpool.tile([128, 128], F32, tag="ident")


            gidx_sb = cpool.tile([128, S_tot], I16, tag="gidx")
            spos_sb = cpool.tile([128, S_tot], I16, tag="spos")
            mask_sb = cpool.tile([64, C2 * 128], F32, tag="mask")
            nc.sync.dma_start(out=ident[:], in_=p_ident[:])
            nc.sync.dma_start(out=gidx_sb[:], in_=p_gidx[:])
            nc.sync.dma_start(out=spos_sb[:], in_=p_spos[:])

            # ---- gathers (SWDGE, queues 0/1); emission interleaved below ----
            gather_insts = []
            scatter_insts = []
            xg_tiles = [None] * n_groups
            scol_acc = 0
            for gi, g in enumerate(groups):
                g["scol"] = scol_acc
                scol_acc += g["C"] * 8

            def emit_gather(gi):
                g = groups[gi]
                C = g["C"]
                if g["quad"]:
                    h_eff = 64
                    in_ap = p_emb[3].rearrange("(q f) h -> q (f h)", f=4)
                else:
                    h_eff = HS[g["cluster"]]
                    cl = g["cluster"]
                    base = CUTOFFS[cl]
                    in_ap = p_emb[cl][g["lo"] - base:g["hi"] - base]
                xg = xgpool.tile([128, C, h_eff], F32, tag=f"xg{gi}")
                gins = nc.gpsimd.dma_gather(
                    out_ap=xg[:], in_ap=in_ap,
                    idxs_ap=gidx_sb[:, g["scol"]:g["scol"] + C * 8],
                    num_idxs=g["mx"], num_idxs_reg=g["mx"],
                    elem_size=h_eff,
                    queue_num=0,
                )
                gather_insts.append(gins.ins)
                xg_tiles[gi] = (xg, h_eff)

            emit_gather(0)
            emit_gather(1)
            emit_gather(2)

            # ---- weights via scalar-engine HWDGE (own ring) + ACT converts ----
            def load_w(dst_bf_ap, src_ap, shape):
                stg = wstg.tile(shape, F32, tag="wstg")
                nc.scalar.dma_start(out=stg[:], in_=src_ap)
                nc.scalar.copy(out=dst_bf_ap, in_=stg[:])

            hwT_r = p_hwT.rearrange("(k p) d -> p k d", p=128)
            w_head = cpool.tile([128, 8, 1024], BF16, tag="w_head")
            for k in range(8):
                load_w(w_head[:, k, :], hwT_r[:, k, :], [128, 1024])
            w_l0 = cpool.tile([128, 2, 1024], BF16, tag="w_l0")
            for k in range(2):
                load_w(w_l0[:, k, :], p_l0.rearrange("(k p) d -> p k d", p=128)[:, k, :], [128, 1024])
            w_l1 = cpool.tile([64, 1024], BF16, tag="w_l1")
            load_w(w_l1[:], p_l1[:], [64, 1024])
            w_l2 = cpool.tile([64, 1024], BF16, tag="w_l2")
            for j in range(4):
                nc.scalar.dma_start(out=p_l2x4[16 * j:16 * j + 16, :], in_=p_l2[:])
            load_w(w_l2[:], p_l2x4[:], [64, 1024])
            nc.sync.dma_start(out=mask_sb[:], in_=p_mask[:])

            def rhs_for(g, k):
                cl = g["cluster"]
                if cl == 0:
                    return lambda sl: w_head[:, k, sl]
                if cl == 1:
                    return lambda sl: w_l0[:, k, sl]
                if cl == 2:
                    return lambda sl: w_l1[:, sl]
                return lambda sl: w_l2[:, sl]

            copy_alt = 0
            stage = None
            chunk_i = 0
            next_gather = 3
            for gi, g in enumerate(groups):
                if next_gather < n_groups:
                    emit_gather(next_gather)
                    next_gather += 1
                xg, h_eff = xg_tiles[gi]
                K = -(-h_eff // 128)
                for c in range(g["C"]):
                    xts = []
                    for k in range(K):
                        kk = min(128, h_eff - 128 * k)
                        tps = pt_pool.tile([128, 128], F32, tag="tps")
                        nc.tensor.transpose(
                            out=tps[:kk, :],
                            in_=xg[:, c, 128 * k:128 * k + kk],
                            identity=ident[:],
                        )
                        xt = xtpool.tile([128, 128], BF16, tag="xt")
                        if g["quad"]:
                            nc.vector.tensor_tensor(
                                out=xt[:kk, :], in0=tps[:kk, :],
                                in1=mask_sb[:, 128 * c:128 * (c + 1)],
                                op=mybir.AluOpType.mult,
                            )
                        else:
                            nc.vector.tensor_copy(out=xt[:kk, :], in_=tps[:kk, :])
                        xts.append((xt, kk))

                    po = po_pool.tile([128, 1024], F32, tag="po")
                    for k, (xt, kk) in enumerate(xts):
                        wk = rhs_for(g, k)
                        for n in range(2):
                            sl = slice(512 * n, 512 * (n + 1))
                            nc.tensor.matmul(
                                out=po[:, sl], lhsT=xt[:kk, :], rhs=wk(sl),
                                start=(k == 0), stop=(k == K - 1),
                            )

                    ck = chunks[chunk_i]
                    slot = c - ck["t0"]
                    if slot == 0:
                        stage = stpool.tile([128, STAGE_TILES, 1024], out_dt, tag="stage")
                    if copy_alt % 2 == 0:
                        nc.vector.tensor_copy(out=stage[:, slot, :], in_=po[:])
                    else:
                        nc.scalar.copy(out=stage[:, slot, :], in_=po[:])
                    copy_alt += 1

                    if slot == ck["ntc"] - 1:
                        col0 = g["scol"] + ck["t0"] * 8
                        sins = nc.gpsimd.dma_scatter_add(
                            out_ap=p_out[chunk_i % N_OUT][:],
                            in_ap=stage[:, :ck["ntc"], :],
                            idxs_ap=spos_sb[:, col0:col0 + ck["ntc"] * 8],
                            num_idxs=ck["n_idx"], num_idxs_reg=ck["n_idx"],
                            elem_size=D,
                            queue_num=1 + chunk_i % 3,
                        )
                        scatter_insts.append(sins.ins)
                        chunk_i += 1

            pass

    nc.compile()
    return nc


_GRAPH_CACHE = {}


def _prepare(tokens_all):
    groups, per_core, chunks = _plan_groups(tokens_all)
    C2 = groups[-1]["C"]

    key = tuple((g["cap"], g["mx"]) for g in groups)
    if key not in _GRAPH_CACHE:
        _GRAPH_CACHE[key] = _build_graph(groups, chunks, C2)
    nc = _GRAPH_CACHE[key]

    gidx_np, spos_np, mask_np, merge_np, cnt_np = [], [], [], [], []
    for i in range(N_CORES):
        gcols, scols = [], []
        cnts = np.zeros((1, 16), np.int32)
        mask = np.zeros((64, C2 * 128), np.float32)
        # which output tensor owns each position (by scatter chunk plan)
        pos_by_out = [[] for _ in range(N_OUT)]
        for gi, g in enumerate(groups):
            sel, loc = per_core[i][gi]
            if g["quad"]:
                gvals = loc // 4
                sub = loc % 4
                for s_i, ssub in enumerate(sub):
                    p, c = s_i % 128, s_i // 128
                    mask[16 * ssub:16 * (ssub + 1), 128 * c + p] = 1.0
            else:
                gvals = loc
            gcols.append(_wrap16(gvals, g["cap"], 0))
            scols.append(_wrap16(sel, g["cap"], TRASH_ROW))
            cnts[0, gi] = len(sel)
        for ci, ck in enumerate(chunks):
            sel = per_core[i][ck["gi"]][0]
            a = 128 * ck["t0"]
            b = min(len(sel), a + ck["n_idx"])
            if b > a:
                pos_by_out[ci % N_OUT].extend(sel[a:b])
        gidx_np.append(np.concatenate(gcols, axis=1))
        spos_np.append(np.concatenate(scols, axis=1))
        mask_np.append(mask)
        merge_np.append([np.asarray(p, np.int64) for p in pos_by_out])
        cnt_np.append(cnts)
    return nc, groups, gidx_np, spos_np, mask_np, merge_np, cnt_np


def run(inputs, trace=False):
    tokens = np.asarray(inputs["tokens"])
    tokens_all = tokens.reshape(N_CORES, N_TOK).astype(np.int64)
    nc, groups, gidx_np, spos_np, mask_np, merge_np, cnt_np = _prepare(tokens_all)

    head_wT = np.ascontiguousarray(np.asarray(inputs["head_w"]).T)
    shared = {
        "head_emb": np.asarray(inputs["head_emb"], np.float32),
        "tail_emb0": np.asarray(inputs["tail_emb0"], np.float32),
        "tail_emb1": np.asarray(inputs["tail_emb1"], np.float32),
        "tail_emb2": np.asarray(inputs["tail_emb2"], np.float32),
        "head_wT": head_wT.astype(np.float32),
        "tail_lin0": np.asarray(inputs["tail_lin0"], np.float32),
        "tail_lin1": np.asarray(inputs["tail_lin1"], np.float32),
        "tail_lin2": np.asarray(inputs["tail_lin2"], np.float32),
    }
    in_maps = []
    for i in range(N_CORES):
        m = dict(shared)
        m["gidx"] = gidx_np[i]
        m["spos"] = spos_np[i]
        m["maskT2"] = mask_np[i]
        m["ident"] = np.eye(128, dtype=np.float32)
        in_maps.append(m)

    res = None
    for attempt in range(3):
        try:
            res = run_bass_kernel_spmd(nc, in_maps, core_ids=list(range(N_CORES)),
                                       trace=trace)
            break
        except Exception:
            if attempt == 2:
                raise
            import time
            time.sleep(2)
    out = np.empty((N_CORES, N_TOK, D), np.float32)
    for i in range(N_CORES):
        for j in range(N_OUT):
            pos = merge_np[i][j]
            if len(pos):
                out[i][pos] = res.results[i][f"out{j}"][pos].astype(np.float32)
    return out, res


def kernel(**inputs):
    out, _ = run(inputs, trace=False)
    return out
